# revision 2
# baseline (speedup 1.0000x reference)
"""Fused multi-head attention (B=4, N=2048, C=1024, H=16, D=64) on 8 NeuronCores.

Sharding: core i handles batch b = i // 2, head-group g = i % 2 (heads
8g..8g+7).  Each core runs an identical Bass/Tile program (SPMD) on its own
input shard:
  - qkv projection for its 1536 features (512 q + 512 k + 512 v), computed
    from host-pretransposed x[b].T and w.T so the contraction dim lands on
    SBUF partitions with contiguous DMA.
  - Q/K are produced directly in [feature, token] layout (what the S matmul
    wants); V in natural [token, feature] layout with an appended ones column
    per head (gives softmax denominators for free in the AV matmul).
  - Attention computes S.T = K.T' @ Q.T per head (scores transposed), exp on
    ScalarE (no max subtraction: |S| <= ~10, well inside fp32/bf16 range),
    AV accumulation in PSUM, then a PE transpose + per-row 1/sum scaling
    produces the output in natural layout.
All matmuls run in bf16 with fp32 PSUM accumulation.
"""

from contextlib import ExitStack

import ml_dtypes
import numpy as np

import concourse.bass as bass
import concourse.mybir as mybir
import concourse.tile as tile
from concourse import bacc
from concourse.bass_utils import run_bass_kernel_spmd
from concourse.masks import make_identity

dt = mybir.dt
AF = mybir.ActivationFunctionType
BF16 = dt.bfloat16
F32 = dt.float32

B, N_TOK, C_IN = 4, 2048, 1024
NH = 8            # heads per core
NPAIR = NH // 2   # heads processed as row-packed pairs in the S matmul
D = 64
WF = 1536         # projected features per core (512 q + 512 k + 512 v)
KC = C_IN // 128  # contraction k-tiles
MT = N_TOK // 128 # token tiles (m / output row chunks)
TB = N_TOK // 512 # 512-wide token blocks for the projection
VROW = 65         # V columns per head incl. ones column


def build_nc(iters: int = 1):
    nc = bacc.Bacc(trn_type="TRN2")
    xT = nc.dram_tensor("xT", [C_IN, N_TOK], BF16, kind="ExternalInput").ap()
    wT = nc.dram_tensor("wT", [C_IN, WF], BF16, kind="ExternalInput").ap()
    qkb = nc.dram_tensor("qkb", [1024], F32, kind="ExternalInput").ap()
    vb = nc.dram_tensor("vb", [512], F32, kind="ExternalInput").ap()
    out = nc.dram_tensor("out", [N_TOK, NH * D], F32, kind="ExternalOutput").ap()

    with tile.TileContext(nc) as tc, ExitStack() as ctx:
        consts = ctx.enter_context(tc.tile_pool(name="consts", bufs=1))
        p_xt = ctx.enter_context(tc.tile_pool(name="p_xt", bufs=KC))
        p_wt = ctx.enter_context(tc.tile_pool(name="p_wt", bufs=KC))
        p_qkt = ctx.enter_context(tc.tile_pool(name="p_qkt", bufs=2 * NPAIR))
        p_vp = ctx.enter_context(tc.tile_pool(name="p_vp", bufs=MT))
        p_pt = ctx.enter_context(tc.tile_pool(name="p_pt", bufs=1))
        p_osb = ctx.enter_context(tc.tile_pool(name="p_osb", bufs=2))
        p_eps = ctx.enter_context(tc.tile_pool(name="p_eps", bufs=4))

        identity = consts.tile([128, 128], BF16, name="identity")
        make_identity(nc, identity)
        qkb_sb = consts.tile([128, 8], F32, name="qkb_sb")
        nc.sync.dma_start(out=qkb_sb, in_=qkb.rearrange("(t p) -> p t", p=128))
        vb_bc = consts.tile([128, 512], F32, name="vb_bc")
        nc.sync.dma_start(
            out=vb_bc,
            in_=bass.AP(tensor=vb.tensor, offset=vb.offset, ap=[[0, 128], vb.ap[0]]),
        )

        def body():
            xt, wt = [], []
            for kc in range(KC):
                tx = p_xt.tile([128, N_TOK], BF16, name=f"xt{kc}", tag="xt")
                nc.sync.dma_start(out=tx, in_=xT[kc * 128 : (kc + 1) * 128, :])
                xt.append(tx)
                tw = p_wt.tile([128, WF], BF16, name=f"wt{kc}", tag="wt")
                nc.sync.dma_start(out=tw, in_=wT[kc * 128 : (kc + 1) * 128, :])
                wt.append(tw)

            qkt = [
                p_qkt.tile([128, N_TOK], BF16, name=f"qkt{ft}", tag="qkt")
                for ft in range(2 * NPAIR)
            ]
            vp = [
                p_vp.tile([128, NH * VROW], BF16, name=f"vp{tt}", tag="vp")
                for tt in range(MT)
            ]

            def proj_qk(ft, pool, tag):
                # qkt[ft] = (x @ w[ft-block].T + b).T  -> [feature, token]
                for tb in range(TB):
                    ps = pool.tile([128, 512], F32, name=f"pj{ft}_{tb}", tag=tag)
                    for kc in range(KC):
                        nc.tensor.matmul(
                            ps,
                            lhsT=wt[kc][:, ft * 128 : (ft + 1) * 128],
                            rhs=xt[kc][:, tb * 512 : (tb + 1) * 512],
                            start=(kc == 0),
                            stop=(kc == KC - 1),
                        )
                    nc.vector.tensor_scalar_add(
                        out=qkt[ft][:, tb * 512 : (tb + 1) * 512],
                        in0=ps,
                        scalar1=qkb_sb[:, ft : ft + 1],
                    )

            def proj_v(tt, pool, tag):
                # vp[tt][:, h*65:h*65+64] = x-tile @ w_v[h].T + b_v[h]; col h*65+64 = 1
                ps = pool.tile([128, 512], F32, name=f"pv{tt}", tag=tag)
                for kc in range(KC):
                    nc.tensor.matmul(
                        ps,
                        lhsT=xt[kc][:, tt * 128 : (tt + 1) * 128],
                        rhs=wt[kc][:, 1024:1536],
                        start=(kc == 0),
                        stop=(kc == KC - 1),
                    )
                t = vp[tt]
                nc.gpsimd.memset(t, 1.0)
                for h in range(NH):
                    nc.vector.tensor_add(
                        out=t[:, h * VROW : h * VROW + 64],
                        in0=ps[:, h * 64 : (h + 1) * 64],
                        in1=vb_bc[:, h * 64 : (h + 1) * 64],
                    )

            # ---- phase A: V projection + first pair's Q/K projection ----
            with tc.tile_pool(name="pp_proj", bufs=4, space="PSUM") as pp_proj:
                for tt in range(MT):
                    proj_v(tt, pp_proj, "pj")
                proj_qk(0, pp_proj, "pj")
                proj_qk(NPAIR, pp_proj, "pj")

            # ---- phase B/C: attention pairs with trickled proj + epilogue ----
            with tc.tile_pool(name="pp_s", bufs=1, space="PSUM") as pp_s, \
                 tc.tile_pool(name="pp_av", bufs=1, space="PSUM") as pp_av, \
                 tc.tile_pool(name="pp_x", bufs=1, space="PSUM") as pp_x:
                for p in range(NPAIR):
                    o_sb = [
                        p_osb.tile([VROW, N_TOK], BF16, name=f"osb{p}_{hh}", tag=f"o{hh}")
                        for hh in range(2)
                    ]
                    for half in range(2):
                        n0 = half * 1024
                        av_a = pp_av.tile([VROW, 1024], F32, name="av_a", tag="av")
                        ptb_list = []
                        for m in range(MT):
                            s_a = pp_s.tile([128, 1024], F32, name="s_a", tag="sA")
                            s_b = pp_s.tile([128, 1024], F32, name="s_b", tag="sB")
                            for nb in range(2):
                                nsl = slice(n0 + nb * 512, n0 + (nb + 1) * 512)
                                nc.tensor.matmul(
                                    s_a[:, nb * 512 : (nb + 1) * 512],
                                    lhsT=qkt[NPAIR + p][0:64, m * 128 : (m + 1) * 128],
                                    rhs=qkt[p][0:64, nsl],
                                    start=True,
                                    stop=True,
                                )
                                nc.tensor.matmul(
                                    s_b[:, nb * 512 : (nb + 1) * 512],
                                    lhsT=qkt[NPAIR + p][64:128, m * 128 : (m + 1) * 128],
                                    rhs=qkt[p][64:128, nsl],
                                    start=True,
                                    stop=True,
                                )
                            pt_a = p_pt.tile([128, 1024], BF16, name="pt_a", tag="ptA", bufs=4)
                            nc.scalar.activation(out=pt_a, in_=s_a, func=AF.Exp, scale=0.125)
                            pt_b = p_pt.tile([128, 1024], BF16, name="pt_b", tag="ptB", bufs=MT + 2)
                            nc.scalar.activation(out=pt_b, in_=s_b, func=AF.Exp, scale=0.125)
                            ptb_list.append((m, pt_b))
                            ha = 2 * p
                            for nb in range(2):
                                nc.tensor.matmul(
                                    av_a[:, nb * 512 : (nb + 1) * 512],
                                    lhsT=vp[m][:, ha * VROW : ha * VROW + VROW],
                                    rhs=pt_a[:, nb * 512 : (nb + 1) * 512],
                                    start=(m == 0),
                                    stop=(m == MT - 1),
                                )
                        nc.vector.tensor_copy(out=o_sb[0][:, n0 : n0 + 1024], in_=av_a)
                        av_b = pp_av.tile([VROW, 1024], F32, name="av_b", tag="av")
                        hb = 2 * p + 1
                        for m, pt_b in ptb_list:
                            for nb in range(2):
                                nc.tensor.matmul(
                                    av_b[:, nb * 512 : (nb + 1) * 512],
                                    lhsT=vp[m][:, hb * VROW : hb * VROW + VROW],
                                    rhs=pt_b[:, nb * 512 : (nb + 1) * 512],
                                    start=(m == 0),
                                    stop=(m == MT - 1),
                                )
                        nc.vector.tensor_copy(out=o_sb[1][:, n0 : n0 + 1024], in_=av_b)

                    # trickle next pair's Q/K projection into PE gaps
                    if p + 1 < NPAIR:
                        proj_qk(p + 1, pp_x, "proj")
                        proj_qk(NPAIR + p + 1, pp_x, "proj")

                    # epilogue: transpose to [token, d], scale rows by 1/sum
                    for hh in range(2):
                        h = 2 * p + hh
                        for chk in range(MT):
                            tr = pp_x.tile([128, VROW], BF16, name="tr", tag="tr")
                            nc.tensor.transpose(
                                tr,
                                in_=o_sb[hh][:, chk * 128 : (chk + 1) * 128],
                                identity=identity[0:VROW, 0:VROW],
                            )
                            rc = p_eps.tile([128, 1], F32, name="rc", tag="rc", bufs=4)
                            nc.vector.reciprocal(out=rc, in_=tr[:, 64:65])
                            ob = p_eps.tile([128, 64], F32, name="ob", tag="ob", bufs=4)
                            nc.vector.tensor_scalar_mul(out=ob, in0=tr[:, 0:64], scalar1=rc)
                            nc.sync.dma_start(
                                out=out[chk * 128 : (chk + 1) * 128, h * 64 : (h + 1) * 64],
                                in_=ob,
                            )

        for _ in range(iters):
            body()

    nc.finalize()
    return nc


_NC_CACHE = {}


def _get_nc(iters: int = 1):
    if iters not in _NC_CACHE:
        _NC_CACHE[iters] = build_nc(iters)
    return _NC_CACHE[iters]


def make_in_maps(x, qkv_w, qkv_b):
    bf = ml_dtypes.bfloat16
    in_maps = []
    for core in range(8):
        b, g = core // 2, core % 2
        xTc = np.ascontiguousarray(x[b].T).astype(bf)
        wq = qkv_w[g * 512 : (g + 1) * 512]
        wk = qkv_w[1024 + g * 512 : 1024 + (g + 1) * 512]
        wv = qkv_w[2048 + g * 512 : 2048 + (g + 1) * 512]
        wTc = np.ascontiguousarray(np.concatenate([wq, wk, wv], axis=0).T).astype(bf)
        qkbc = np.ascontiguousarray(
            np.concatenate(
                [qkv_b[g * 512 : (g + 1) * 512], qkv_b[1024 + g * 512 : 1024 + (g + 1) * 512]]
            )
        ).astype(np.float32)
        vbc = np.ascontiguousarray(wv_bias := qkv_b[2048 + g * 512 : 2048 + (g + 1) * 512]).astype(
            np.float32
        )
        in_maps.append({"xT": xTc, "wT": wTc, "qkb": qkbc, "vb": vbc})
    return in_maps


def kernel(x, qkv_w, qkv_b):
    x = np.asarray(x, dtype=np.float32)
    qkv_w = np.asarray(qkv_w, dtype=np.float32)
    qkv_b = np.asarray(qkv_b, dtype=np.float32)
    nc = _get_nc(1)
    in_maps = make_in_maps(x, qkv_w, qkv_b)
    res = run_bass_kernel_spmd(nc, in_maps, core_ids=list(range(8)))
    full = np.empty((B, N_TOK, C_IN), dtype=np.float32)
    for core in range(8):
        b, g = core // 2, core % 2
        full[b, :, g * 512 : (g + 1) * 512] = res.results[core]["out"]
    return full


# revision 9
# speedup vs baseline: 351.6224x; 351.6224x over previous
"""Fused multi-head attention (B=4, N=2048, C=1024, H=16, D=64) on 8 NeuronCores.

Sharding: core i handles batch b = i // 2, head-group g = i % 2 (heads
8g..8g+7).  Each core runs an identical Bass/Tile program (SPMD) on its own
input shard:
  - qkv projection for its 1536 features (512 q + 512 k + 512 v), computed
    from host-pretransposed x[b].T and w.T so the contraction dim lands on
    SBUF partitions with contiguous DMA.
  - Q/K are produced directly in [feature, token] layout (what the S matmul
    wants); V in natural [token, feature] layout with an appended ones column
    per head (gives softmax denominators for free in the AV matmul).
  - Attention computes S.T = K.T' @ Q.T per head (scores transposed), exp on
    ScalarE (no max subtraction: |S| <= ~10, well inside fp32/bf16 range),
    AV accumulation in PSUM, then a PE transpose + per-row 1/sum scaling
    produces the output in natural layout.
All matmuls run in bf16 with fp32 PSUM accumulation.
"""

from contextlib import ExitStack

import ml_dtypes
import numpy as np

import concourse.bass as bass
import concourse.mybir as mybir
import concourse.tile as tile
from concourse import bacc
from concourse.bass_utils import run_bass_kernel_spmd
from concourse.masks import make_identity

dt = mybir.dt
AF = mybir.ActivationFunctionType
BF16 = dt.bfloat16
F32 = dt.float32

B, N_TOK, C_IN = 4, 2048, 1024
NH = 8            # heads per core
NPAIR = NH // 2   # heads processed as row-packed pairs in the S matmul
D = 64
WF = 1536         # projected features per core (512 q + 512 k + 512 v)
KC = C_IN // 128  # contraction k-tiles
MT = N_TOK // 128 # token tiles (m / output row chunks)
TB = N_TOK // 512 # 512-wide token blocks for the projection
VROW = 65         # V columns per head incl. ones column


def build_nc(iters: int = 1):
    nc = bacc.Bacc(trn_type="TRN2")
    xT = nc.dram_tensor("xT", [C_IN, N_TOK], BF16, kind="ExternalInput").ap()
    wT = nc.dram_tensor("wT", [C_IN, WF], BF16, kind="ExternalInput").ap()
    qkb = nc.dram_tensor("qkb", [1024], F32, kind="ExternalInput").ap()
    vb = nc.dram_tensor("vb", [512], F32, kind="ExternalInput").ap()
    out = nc.dram_tensor("out", [N_TOK, NH * D], F32, kind="ExternalOutput").ap()

    with tile.TileContext(nc) as tc, ExitStack() as ctx:
        consts = ctx.enter_context(tc.tile_pool(name="consts", bufs=1))
        p_xt = ctx.enter_context(tc.tile_pool(name="p_xt", bufs=KC))
        p_wt = ctx.enter_context(tc.tile_pool(name="p_wt", bufs=KC))
        p_qkt = ctx.enter_context(tc.tile_pool(name="p_qkt", bufs=2 * NPAIR))
        p_vp = ctx.enter_context(tc.tile_pool(name="p_vp", bufs=MT))
        p_pt = ctx.enter_context(tc.tile_pool(name="p_pt", bufs=1))
        p_osb = ctx.enter_context(tc.tile_pool(name="p_osb", bufs=2))
        p_eps = ctx.enter_context(tc.tile_pool(name="p_eps", bufs=4))

        identity = consts.tile([128, 128], BF16, name="identity")
        make_identity(nc, identity)
        qkb_sb = consts.tile([128, 8], F32, name="qkb_sb")
        nc.sync.dma_start(out=qkb_sb, in_=qkb.rearrange("(t p) -> p t", p=128))
        vb_bc = consts.tile([128, 512], F32, name="vb_bc")
        nc.sync.dma_start(
            out=vb_bc,
            in_=bass.AP(tensor=vb.tensor, offset=vb.offset, ap=[[0, 128], vb.ap[0]]),
        )

        def body():
            xt, wt = [], []
            for kc in range(KC):
                tx = p_xt.tile([128, N_TOK], BF16, name=f"xt{kc}", tag="xt")
                nc.sync.dma_start(out=tx, in_=xT[kc * 128 : (kc + 1) * 128, :])
                xt.append(tx)
                tw = p_wt.tile([128, WF], BF16, name=f"wt{kc}", tag="wt")
                nc.sync.dma_start(out=tw, in_=wT[kc * 128 : (kc + 1) * 128, :])
                wt.append(tw)

            qkt = [
                p_qkt.tile([128, N_TOK], BF16, name=f"qkt{ft}", tag="qkt")
                for ft in range(2 * NPAIR)
            ]
            vp = [
                p_vp.tile([128, NH * VROW], BF16, name=f"vp{tt}", tag="vp")
                for tt in range(MT)
            ]

            def proj_qk(ft, pool, tag):
                # qkt[ft] = (x @ w[ft-block].T + b).T  -> [feature, token]
                for tb in range(TB):
                    ps = pool.tile([128, 512], F32, name=f"pj{ft}_{tb}", tag=tag)
                    for kc in range(KC):
                        nc.tensor.matmul(
                            ps,
                            lhsT=wt[kc][:, ft * 128 : (ft + 1) * 128],
                            rhs=xt[kc][:, tb * 512 : (tb + 1) * 512],
                            start=(kc == 0),
                            stop=(kc == KC - 1),
                        )
                    nc.vector.tensor_scalar_add(
                        out=qkt[ft][:, tb * 512 : (tb + 1) * 512],
                        in0=ps,
                        scalar1=qkb_sb[:, ft : ft + 1],
                    )

            def proj_v(tt, pool, tag):
                # vp[tt][:, h*65:h*65+64] = x-tile @ w_v[h].T + b_v[h]; col h*65+64 = 1
                ps = pool.tile([128, 512], F32, name=f"pv{tt}", tag=tag)
                for kc in range(KC):
                    nc.tensor.matmul(
                        ps,
                        lhsT=xt[kc][:, tt * 128 : (tt + 1) * 128],
                        rhs=wt[kc][:, 1024:1536],
                        start=(kc == 0),
                        stop=(kc == KC - 1),
                    )
                t = vp[tt]
                nc.gpsimd.memset(t, 1.0)
                for h in range(NH):
                    nc.vector.tensor_add(
                        out=t[:, h * VROW : h * VROW + 64],
                        in0=ps[:, h * 64 : (h + 1) * 64],
                        in1=vb_bc[:, h * 64 : (h + 1) * 64],
                    )

            # ---- phase A: first pair's Q/K projection, then V projection ----
            with tc.tile_pool(name="pp_proj", bufs=4, space="PSUM") as pp_proj:
                proj_qk(0, pp_proj, "pj")
                proj_qk(NPAIR, pp_proj, "pj")
                for tt in range(MT):
                    proj_v(tt, pp_proj, "pj")

            # ---- phase B/C: attention pairs with trickled proj + epilogue ----
            # PSUM budget (8 banks): sA 2 + sB 2 + avA 2 + b0 1 + b1 1.
            # Head B accumulates inline into two single-bank tiles (b0/b1) so
            # no slot ring couples head A's and head B's accumulators; the
            # trickled projection rides the b0/b1 rings between pairs and the
            # epilogue transposes ride the avA ring.
            with tc.tile_pool(name="pp_s", bufs=1, space="PSUM") as pp_s, \
                 tc.tile_pool(name="pp_av", bufs=1, space="PSUM") as pp_av, \
                 tc.tile_pool(name="pp_b", bufs=1, space="PSUM") as pp_b:

                def epilogue_half(o_t, h, half):
                    # transpose each 128-token chunk to [token, d'], then rows /= sums
                    for chk in range(8 * half, 8 * (half + 1)):
                        tr = pp_av.tile([128, VROW], BF16, name="tr", tag="av")
                        nc.tensor.transpose(
                            tr,
                            in_=o_t[:, chk * 128 : (chk + 1) * 128],
                            identity=identity[0:VROW, 0:VROW],
                        )
                        rc = p_eps.tile([128, 1], F32, name="rc", tag="rc", bufs=4)
                        nc.vector.reciprocal(out=rc, in_=tr[:, 64:65])
                        ob = p_eps.tile([128, 64], F32, name="ob", tag="ob", bufs=4)
                        nc.vector.tensor_scalar_mul(out=ob, in0=tr[:, 0:64], scalar1=rc)
                        nc.sync.dma_start(
                            out=out[chk * 128 : (chk + 1) * 128, h * 64 : (h + 1) * 64],
                            in_=ob,
                        )

                epilogues = []
                for p in range(NPAIR):
                    o_sb = [
                        p_osb.tile([VROW, N_TOK], BF16, name=f"osb{p}_{hh}", tag=f"o{hh}")
                        for hh in range(2)
                    ]
                    ha, hb = 2 * p, 2 * p + 1
                    for half in range(2):
                        n0 = half * 1024
                        av_a = pp_av.tile([VROW, 1024], F32, name="av_a", tag="av")
                        av_b = [
                            pp_b.tile([VROW, 512], F32, name=f"av_b{nb}", tag=f"b{nb}")
                            for nb in range(2)
                        ]
                        for m in range(MT):
                            s_a = pp_s.tile([128, 1024], F32, name="s_a", tag="sA")
                            s_b = pp_s.tile([128, 1024], F32, name="s_b", tag="sB")
                            for nb in range(2):
                                nsl = slice(n0 + nb * 512, n0 + (nb + 1) * 512)
                                nc.tensor.matmul(
                                    s_a[:, nb * 512 : (nb + 1) * 512],
                                    lhsT=qkt[NPAIR + p][0:64, m * 128 : (m + 1) * 128],
                                    rhs=qkt[p][0:64, nsl],
                                    start=True,
                                    stop=True,
                                )
                                nc.tensor.matmul(
                                    s_b[:, nb * 512 : (nb + 1) * 512],
                                    lhsT=qkt[NPAIR + p][64:128, m * 128 : (m + 1) * 128],
                                    rhs=qkt[p][64:128, nsl],
                                    start=True,
                                    stop=True,
                                )
                            pt_a = p_pt.tile([128, 1024], BF16, name="pt_a", tag="ptA", bufs=8)
                            nc.scalar.activation(out=pt_a, in_=s_a, func=AF.Exp, scale=0.125)
                            pt_b = p_pt.tile([128, 1024], BF16, name="pt_b", tag="ptB", bufs=8)
                            nc.scalar.activation(out=pt_b, in_=s_b, func=AF.Exp, scale=0.125)
                            for nb in range(2):
                                nc.tensor.matmul(
                                    av_a[:, nb * 512 : (nb + 1) * 512],
                                    lhsT=vp[m][:, ha * VROW : ha * VROW + VROW],
                                    rhs=pt_a[:, nb * 512 : (nb + 1) * 512],
                                    start=(m == 0),
                                    stop=(m == MT - 1),
                                )
                                nc.tensor.matmul(
                                    av_b[nb],
                                    lhsT=vp[m][:, hb * VROW : hb * VROW + VROW],
                                    rhs=pt_b[:, nb * 512 : (nb + 1) * 512],
                                    start=(m == 0),
                                    stop=(m == MT - 1),
                                )
                        nc.vector.tensor_copy(out=o_sb[0][:, n0 : n0 + 1024], in_=av_a)
                        for nb in range(2):
                            nc.vector.tensor_copy(
                                out=o_sb[1][:, n0 + nb * 512 : n0 + (nb + 1) * 512],
                                in_=av_b[nb],
                            )
                        for hh in range(2):
                            epilogue_half(o_sb[hh], 2 * p + hh, half)

                    # trickle next pair's Q/K projection into PE gaps (psum via
                    # the b0/b1 rings, which are idle between accumulations)
                    if p + 1 < NPAIR:
                        proj_qk(p + 1, pp_b, "b0")
                        proj_qk(NPAIR + p + 1, pp_b, "b1")

        for _ in range(iters):
            body()

    nc.finalize()
    return nc


_NC_CACHE = {}


def _get_nc(iters: int = 1):
    if iters not in _NC_CACHE:
        _NC_CACHE[iters] = build_nc(iters)
    return _NC_CACHE[iters]


def make_in_maps(x, qkv_w, qkv_b):
    bf = ml_dtypes.bfloat16
    in_maps = []
    for core in range(8):
        b, g = core // 2, core % 2
        xTc = np.ascontiguousarray(x[b].T).astype(bf)
        wq = qkv_w[g * 512 : (g + 1) * 512]
        wk = qkv_w[1024 + g * 512 : 1024 + (g + 1) * 512]
        wv = qkv_w[2048 + g * 512 : 2048 + (g + 1) * 512]
        wTc = np.ascontiguousarray(np.concatenate([wq, wk, wv], axis=0).T).astype(bf)
        qkbc = np.ascontiguousarray(
            np.concatenate(
                [qkv_b[g * 512 : (g + 1) * 512], qkv_b[1024 + g * 512 : 1024 + (g + 1) * 512]]
            )
        ).astype(np.float32)
        vbc = np.ascontiguousarray(wv_bias := qkv_b[2048 + g * 512 : 2048 + (g + 1) * 512]).astype(
            np.float32
        )
        in_maps.append({"xT": xTc, "wT": wTc, "qkb": qkbc, "vb": vbc})
    return in_maps


def kernel(x, qkv_w, qkv_b):
    x = np.asarray(x, dtype=np.float32)
    qkv_w = np.asarray(qkv_w, dtype=np.float32)
    qkv_b = np.asarray(qkv_b, dtype=np.float32)
    nc = _get_nc(1)
    in_maps = make_in_maps(x, qkv_w, qkv_b)
    res = run_bass_kernel_spmd(nc, in_maps, core_ids=list(range(8)))
    full = np.empty((B, N_TOK, C_IN), dtype=np.float32)
    for core in range(8):
        b, g = core // 2, core % 2
        full[b, :, g * 512 : (g + 1) * 512] = res.results[core]["out"]
    return full


# revision 10
# speedup vs baseline: 641.4167x; 1.8242x over previous
"""Fused multi-head attention (B=4, N=2048, C=1024, H=16, D=64) on 8 NeuronCores.

Sharding: core i handles batch b = i // 2, head-group g = i % 2 (heads
8g..8g+7).  Each core runs an identical Bass/Tile program (SPMD) on its own
input shard:
  - qkv projection for its 1536 features (512 q + 512 k + 512 v), computed
    from host-pretransposed x[b].T and w.T so the contraction dim lands on
    SBUF partitions with contiguous DMA.
  - Q/K are produced directly in [feature, token] layout (what the S matmul
    wants); V in natural [token, feature] layout with an appended ones column
    per head (gives softmax denominators for free in the AV matmul).
  - Attention computes S.T = K.T' @ Q.T per head (scores transposed), exp on
    ScalarE (no max subtraction: |S| <= ~10, well inside fp32/bf16 range),
    AV accumulation in PSUM, then a PE transpose + per-row 1/sum scaling
    produces the output in natural layout.
All matmuls run in bf16 with fp32 PSUM accumulation.
"""

from contextlib import ExitStack

import ml_dtypes
import numpy as np

import concourse.bass as bass
import concourse.mybir as mybir
import concourse.tile as tile
from concourse import bacc
from concourse.bass_utils import run_bass_kernel_spmd
from concourse.masks import make_identity

dt = mybir.dt
AF = mybir.ActivationFunctionType
BF16 = dt.bfloat16
F32 = dt.float32

B, N_TOK, C_IN = 4, 2048, 1024
NH = 8            # heads per core
NPAIR = NH // 2   # heads processed as row-packed pairs in the S matmul
D = 64
WF = 1536         # projected features per core (512 q + 512 k + 512 v)
KC = C_IN // 128  # contraction k-tiles
MT = N_TOK // 128 # token tiles (m / output row chunks)
TB = N_TOK // 512 # 512-wide token blocks for the projection
VROW = 65         # V columns per head incl. ones column


def build_nc(iters: int = 1):
    nc = bacc.Bacc(trn_type="TRN2")
    xT = nc.dram_tensor("xT", [C_IN, N_TOK], BF16, kind="ExternalInput").ap()
    wT = nc.dram_tensor("wT", [C_IN, WF], BF16, kind="ExternalInput").ap()
    qkb = nc.dram_tensor("qkb", [1024], F32, kind="ExternalInput").ap()
    vb = nc.dram_tensor("vb", [512], F32, kind="ExternalInput").ap()
    out = nc.dram_tensor("out", [N_TOK, NH * D], F32, kind="ExternalOutput").ap()

    with tile.TileContext(nc) as tc, ExitStack() as ctx:
        consts = ctx.enter_context(tc.tile_pool(name="consts", bufs=1))
        p_xt = ctx.enter_context(tc.tile_pool(name="p_xt", bufs=KC))
        p_wt = ctx.enter_context(tc.tile_pool(name="p_wt", bufs=KC))
        p_qkt = ctx.enter_context(tc.tile_pool(name="p_qkt", bufs=2 * NPAIR))
        p_vp = ctx.enter_context(tc.tile_pool(name="p_vp", bufs=MT))
        p_pt = ctx.enter_context(tc.tile_pool(name="p_pt", bufs=1))
        p_osb = ctx.enter_context(tc.tile_pool(name="p_osb", bufs=2))
        p_eps = ctx.enter_context(tc.tile_pool(name="p_eps", bufs=4))

        identity = consts.tile([128, 128], BF16, name="identity")
        make_identity(nc, identity)
        qkb_sb = consts.tile([128, 8], F32, name="qkb_sb")
        nc.sync.dma_start(out=qkb_sb, in_=qkb.rearrange("(t p) -> p t", p=128))
        vb_bc = consts.tile([128, 512], F32, name="vb_bc")
        nc.sync.dma_start(
            out=vb_bc,
            in_=bass.AP(tensor=vb.tensor, offset=vb.offset, ap=[[0, 128], vb.ap[0]]),
        )

        def body():
            xt, wt = [], []
            for kc in range(KC):
                tx = p_xt.tile([128, N_TOK], BF16, name=f"xt{kc}", tag="xt")
                nc.sync.dma_start(out=tx, in_=xT[kc * 128 : (kc + 1) * 128, :])
                xt.append(tx)
                tw = p_wt.tile([128, WF], BF16, name=f"wt{kc}", tag="wt")
                nc.sync.dma_start(out=tw, in_=wT[kc * 128 : (kc + 1) * 128, :])
                wt.append(tw)

            qkt = [
                p_qkt.tile([128, N_TOK], BF16, name=f"qkt{ft}", tag="qkt")
                for ft in range(2 * NPAIR)
            ]
            vp = [
                p_vp.tile([128, NH * VROW], BF16, name=f"vp{tt}", tag="vp")
                for tt in range(MT)
            ]

            def proj_qk(ft, pool, tag):
                # qkt[ft] = (x @ w[ft-block].T + b).T  -> [feature, token]
                for tb in range(TB):
                    ps = pool.tile([128, 512], F32, name=f"pj{ft}_{tb}", tag=tag)
                    for kc in range(KC):
                        nc.tensor.matmul(
                            ps,
                            lhsT=wt[kc][:, ft * 128 : (ft + 1) * 128],
                            rhs=xt[kc][:, tb * 512 : (tb + 1) * 512],
                            start=(kc == 0),
                            stop=(kc == KC - 1),
                        )
                    nc.vector.tensor_scalar_add(
                        out=qkt[ft][:, tb * 512 : (tb + 1) * 512],
                        in0=ps,
                        scalar1=qkb_sb[:, ft : ft + 1],
                    )

            def proj_v(tt, pool, tag):
                # vp[tt][:, h*65:h*65+64] = x-tile @ w_v[h].T + b_v[h]; col h*65+64 = 1
                ps = pool.tile([128, 512], F32, name=f"pv{tt}", tag=tag)
                for kc in range(KC):
                    nc.tensor.matmul(
                        ps,
                        lhsT=xt[kc][:, tt * 128 : (tt + 1) * 128],
                        rhs=wt[kc][:, 1024:1536],
                        start=(kc == 0),
                        stop=(kc == KC - 1),
                    )
                t = vp[tt]
                nc.gpsimd.memset(t, 1.0)
                for h in range(NH):
                    nc.vector.tensor_add(
                        out=t[:, h * VROW : h * VROW + 64],
                        in0=ps[:, h * 64 : (h + 1) * 64],
                        in1=vb_bc[:, h * 64 : (h + 1) * 64],
                    )

            # ---- phase A: first pair's Q/K projection, then V projection ----
            with tc.tile_pool(name="pp_proj", bufs=4, space="PSUM") as pp_proj:
                proj_qk(0, pp_proj, "pj")
                proj_qk(NPAIR, pp_proj, "pj")
                for tt in range(MT):
                    proj_v(tt, pp_proj, "pj")

            # ---- phase B/C: attention pairs with trickled proj + epilogue ----
            # PSUM budget (8 banks): sA 2 + sB 2 + avA 2 + b0 1 + b1 1.
            # Head B accumulates inline into two single-bank tiles (b0/b1) so
            # no slot ring couples head A's and head B's accumulators; the
            # trickled projection rides the b0/b1 rings between pairs and the
            # epilogue transposes ride the avA ring.
            with tc.tile_pool(name="pp_s", bufs=1, space="PSUM") as pp_s, \
                 tc.tile_pool(name="pp_av", bufs=1, space="PSUM") as pp_av, \
                 tc.tile_pool(name="pp_b", bufs=1, space="PSUM") as pp_b:

                def epilogue_half(o_t, h, half):
                    # transpose each 128-token chunk to [token, d'], then rows /= sums
                    for chk in range(8 * half, 8 * (half + 1)):
                        tr = pp_av.tile([128, VROW], BF16, name="tr", tag="av")
                        nc.tensor.transpose(
                            tr,
                            in_=o_t[:, chk * 128 : (chk + 1) * 128],
                            identity=identity[0:VROW, 0:VROW],
                        )
                        rc = p_eps.tile([128, 1], F32, name="rc", tag="rc", bufs=4)
                        nc.vector.reciprocal(out=rc, in_=tr[:, 64:65])
                        ob = p_eps.tile([128, 64], F32, name="ob", tag="ob", bufs=4)
                        nc.vector.tensor_scalar_mul(out=ob, in0=tr[:, 0:64], scalar1=rc)
                        nc.sync.dma_start(
                            out=out[chk * 128 : (chk + 1) * 128, h * 64 : (h + 1) * 64],
                            in_=ob,
                        )

                epilogues = []
                for p in range(NPAIR):
                    o_sb = [
                        p_osb.tile([VROW, N_TOK], BF16, name=f"osb{p}_{hh}", tag=f"o{hh}")
                        for hh in range(2)
                    ]
                    ha, hb = 2 * p, 2 * p + 1
                    for half in range(2):
                        n0 = half * 1024
                        av_a = pp_av.tile([VROW, 1024], F32, name="av_a", tag="av")
                        av_b = [
                            pp_b.tile([VROW, 512], F32, name=f"av_b{nb}", tag=f"b{nb}")
                            for nb in range(2)
                        ]
                        for m in range(MT):
                            s_a = pp_s.tile([128, 1024], F32, name="s_a", tag="sA")
                            s_b = pp_s.tile([128, 1024], F32, name="s_b", tag="sB")
                            for nb in range(2):
                                nsl = slice(n0 + nb * 512, n0 + (nb + 1) * 512)
                                nc.tensor.matmul(
                                    s_a[:, nb * 512 : (nb + 1) * 512],
                                    lhsT=qkt[NPAIR + p][0:64, m * 128 : (m + 1) * 128],
                                    rhs=qkt[p][0:64, nsl],
                                    start=True,
                                    stop=True,
                                )
                                nc.tensor.matmul(
                                    s_b[:, nb * 512 : (nb + 1) * 512],
                                    lhsT=qkt[NPAIR + p][64:128, m * 128 : (m + 1) * 128],
                                    rhs=qkt[p][64:128, nsl],
                                    start=True,
                                    stop=True,
                                )
                            pt_a = p_pt.tile([128, 1024], BF16, name="pt_a", tag="ptA", bufs=8)
                            nc.scalar.activation(out=pt_a, in_=s_a, func=AF.Exp, scale=0.125)
                            pt_b = p_pt.tile([128, 1024], BF16, name="pt_b", tag="ptB", bufs=8)
                            nc.scalar.activation(out=pt_b, in_=s_b, func=AF.Exp, scale=0.125)
                            for nb in range(2):
                                nc.tensor.matmul(
                                    av_a[:, nb * 512 : (nb + 1) * 512],
                                    lhsT=vp[m][:, ha * VROW : ha * VROW + VROW],
                                    rhs=pt_a[:, nb * 512 : (nb + 1) * 512],
                                    start=(m == 0),
                                    stop=(m == MT - 1),
                                )
                                nc.tensor.matmul(
                                    av_b[nb],
                                    lhsT=vp[m][:, hb * VROW : hb * VROW + VROW],
                                    rhs=pt_b[:, nb * 512 : (nb + 1) * 512],
                                    start=(m == 0),
                                    stop=(m == MT - 1),
                                )
                        nc.vector.tensor_copy(out=o_sb[0][:, n0 : n0 + 1024], in_=av_a)
                        for nb in range(2):
                            nc.vector.tensor_copy(
                                out=o_sb[1][:, n0 + nb * 512 : n0 + (nb + 1) * 512],
                                in_=av_b[nb],
                            )
                        for hh in range(2):
                            epilogue_half(o_sb[hh], 2 * p + hh, half)

                    # trickle next pair's Q/K projection into PE gaps (psum via
                    # the b0/b1 rings, which are idle between accumulations)
                    if p + 1 < NPAIR:
                        proj_qk(p + 1, pp_b, "b0")
                        proj_qk(NPAIR + p + 1, pp_b, "b1")

        for _ in range(iters):
            body()

    nc.finalize()
    return nc


_NC_CACHE = {}


def _get_nc(iters: int = 1):
    if iters not in _NC_CACHE:
        _NC_CACHE[iters] = build_nc(iters)
    return _NC_CACHE[iters]


def make_in_maps(x, qkv_w, qkv_b):
    bf = ml_dtypes.bfloat16
    in_maps = []
    for core in range(8):
        b, g = core // 2, core % 2
        xTc = np.ascontiguousarray(x[b].T).astype(bf)
        wq = qkv_w[g * 512 : (g + 1) * 512]
        wk = qkv_w[1024 + g * 512 : 1024 + (g + 1) * 512]
        wv = qkv_w[2048 + g * 512 : 2048 + (g + 1) * 512]
        wTc = np.ascontiguousarray(np.concatenate([wq, wk, wv], axis=0).T).astype(bf)
        qkbc = np.ascontiguousarray(
            np.concatenate(
                [qkv_b[g * 512 : (g + 1) * 512], qkv_b[1024 + g * 512 : 1024 + (g + 1) * 512]]
            )
        ).astype(np.float32)
        vbc = np.ascontiguousarray(qkv_b[2048 + g * 512 : 2048 + (g + 1) * 512]).astype(
            np.float32
        )
        in_maps.append({"xT": xTc, "wT": wTc, "qkb": qkbc, "vb": vbc})
    return in_maps


_RUNNER_CACHE = {}


def _get_runner(iters: int = 1, n_cores: int = 8):
    """Build the shard_map-wrapped bass_exec executable once and reuse it, so
    repeated kernel() calls don't re-ship the NEFF through the axon tunnel."""
    if iters in _RUNNER_CACHE:
        return _RUNNER_CACHE[iters]
    import jax
    from jax.sharding import Mesh, PartitionSpec
    from jax.experimental.shard_map import shard_map
    from concourse.bass2jax import (
        _bass_exec_p,
        install_neuronx_cc_hook,
        partition_id_tensor,
    )

    nc = _get_nc(iters)
    install_neuronx_cc_hook()
    partition_name = nc.partition_id_tensor.name if nc.partition_id_tensor else None
    in_names, out_names, out_avals, zero_outs = [], [], [], []
    for alloc in nc.m.functions[0].allocations:
        if not isinstance(alloc, mybir.MemoryLocationSet):
            continue
        name = alloc.memorylocations[0].name
        if alloc.kind == "ExternalInput":
            if name != partition_name:
                in_names.append(name)
        elif alloc.kind == "ExternalOutput":
            shape = tuple(alloc.tensor_shape)
            npdt = dt.np(alloc.dtype)
            out_names.append(name)
            out_avals.append(jax.core.ShapedArray(shape, npdt))
            zero_outs.append(np.zeros(shape, npdt))
    n_params = len(in_names)
    all_in_names = list(in_names) + list(out_names)
    if partition_name is not None:
        all_in_names.append(partition_name)

    def _body(*args):
        operands = list(args)
        if partition_name is not None:
            operands.append(partition_id_tensor())
        return tuple(
            _bass_exec_p.bind(
                *operands,
                out_avals=tuple(out_avals),
                in_names=tuple(all_in_names),
                out_names=tuple(out_names),
                lowering_input_output_aliases=(),
                sim_require_finite=True,
                sim_require_nnan=True,
                nc=nc,
            )
        )

    devices = jax.devices()[:n_cores]
    mesh = Mesh(np.asarray(devices), ("core",))
    in_specs = (PartitionSpec("core"),) * (n_params + len(out_names))
    out_specs = (PartitionSpec("core"),) * len(out_names)
    fn = jax.jit(
        shard_map(_body, mesh=mesh, in_specs=in_specs, out_specs=out_specs, check_rep=False)
    )
    zero_concat = [
        np.zeros((n_cores * z.shape[0], *z.shape[1:]), z.dtype) for z in zero_outs
    ]
    _RUNNER_CACHE[iters] = (fn, in_names, zero_concat, mesh)
    return _RUNNER_CACHE[iters]


def kernel(x, qkv_w, qkv_b):
    import jax

    x = np.asarray(x, dtype=np.float32)
    qkv_w = np.asarray(qkv_w, dtype=np.float32)
    qkv_b = np.asarray(qkv_b, dtype=np.float32)
    in_maps = make_in_maps(x, qkv_w, qkv_b)
    fn, in_names, zero_concat, _ = _get_runner(1)
    concat_in = [
        np.concatenate([in_maps[c][name] for c in range(8)], axis=0) for name in in_names
    ]
    outs = fn(*concat_in, *zero_concat)
    out_global = np.asarray(jax.block_until_ready(outs)[0])
    full = np.empty((B, N_TOK, C_IN), dtype=np.float32)
    for core in range(8):
        b, g = core // 2, core % 2
        full[b, :, g * 512 : (g + 1) * 512] = out_global[core * N_TOK : (core + 1) * N_TOK]
    return full
